# revision 29
# baseline (speedup 1.0000x reference)
"""AttentionalJoin kernel for 8 Trainium2 NeuronCores.

Math: the reference builds full (M x M) self-attention over M = N+1 tokens
(CLS prepended) but returns only the CLS row of the projected output.  Only
the CLS query survives, so attention collapses to a softmax-weighted token
pooling:

    q       = Wq @ cls                       (per head h: q_h)
    score_t = scale * q_h . (Wk x_t)_h  =  x_t . R[:, h],   R = scale*Wk_h^T q_h
    p       = softmax over the M tokens (scores bounded ~[-6, 6]; no max-sub)
    pooled_h = sum_t p_t x_t                 (linearity: project AFTER pooling)
    out     = proj( concat_h Wv_h pooled_h ) + proj_b

Device pipeline, software-pipelined at depth 3 over 512-token chunks:
  T(i)   : 16 identity matmuls build X^T per c-chunk (x-block as fp16 FWL
           weights, N=128); PSUM->SBUF drains split 2:1 between DVE and ACT
  S(i-1) : scores TRANSPOSED: per t-block a 4-matmul chain over c-chunks
           with the X^T block as weights and R_q as the 8-col moving
           operand -> [t,h] in PSUM; one ACT exp yields E^T directly (no
           E-transpose pass) and one tiny matmul against a ones vector
           forms the per-chunk Z partition sums
  P(i-2) : 16 matmuls x_block.T @ E^T -> pooled^T [c128, h], one PSUM
           accumulation group per batch across all its chunks
x streams from HBM once in fp16 (rel err ~3e-4); the first chunks are
DMA'd in 128KB j-pieces issued alternately from SP/ACT HWDGE so the PE
starts early, and a burst of heater matmuls keeps the PE clock-gate
warming while they land.  The tiny tail (CLS term, 1/Z, head-mix, proj,
bias) runs on host.

Sharding: data-parallel over the batch dim, 2 batches per core.
"""

import numpy as np

H = 8
C = 512
HD = C // H
B = 16
N = 2048
NCORES = 8
BPC = B // NCORES          # batches per core
TOK = BPC * N              # tokens per core (4096)
NCHUNK = TOK // 512        # 512-token chunks per core (8; 4 per batch)
CPB = NCHUNK // BPC        # chunks per batch (4)
NSPLIT = 3                 # leading chunks DMA'd in j-pieces for early start
MAX_DRAIN_WAITS = 1        # this walrus rejects instructions w/ >1 sem wait

_cached = {}


def _patch_drain():
    """The container's walrus codegen rejects instructions carrying more
    than one sem wait ("Too many sync wait commands").  Split extra waits
    onto dedicated same-engine NOPs, which preserves semantics (engine
    queues are in-order)."""
    import concourse.tile as tile_mod
    from concourse import mybir
    from bass_rust import ScopedClock

    if getattr(tile_mod.TileContext, "_drain_patched", False):
        return

    orig_lower = tile_mod.TileContext._lower_ordered_insts

    def _lower_ordered_insts(self, ordered):
        nc = self.nc
        for bbname, insts in ordered.items():
            out = []
            for inst in insts:
                si = inst.sync_info
                if si is not None and si.on_wait and len(si.on_wait) > MAX_DRAIN_WAITS:
                    waits = list(si.on_wait)
                    extra, keep = waits[:-MAX_DRAIN_WAITS], waits[-MAX_DRAIN_WAITS:]
                    for w in extra:
                        nop = mybir.InstNoOp(
                            name=f"waitsplit-{nc.next_id()}",
                            engine=inst.engine,
                            ins=[],
                            outs=[],
                            bass_nofuse=True,
                            sync_info=mybir.SyncInfo(on_wait=[w], on_update=[]),
                            debug=inst.debug,
                        )
                        out.append(nop)
                    inst.sync_info = mybir.SyncInfo(
                        on_wait=keep, on_update=list(si.on_update)
                    )
                out.append(inst)
            ordered[bbname] = out
        return orig_lower(self, ordered)

    tile_mod.TileContext._lower_ordered_insts = _lower_ordered_insts

    def _drain_and_barrier(self, tick_clock, wait_clock):
        nc = self.nc
        probe = mybir.InstNoOp(
            name=f"drain-wait-probe-{nc.next_id()}",
            engine=mybir.EngineType.SP,
            ins=[],
            outs=[],
        )
        wait_clock.add_sem_waits(probe, ScopedClock({None: tick_clock.global_clock}))
        waits = list(probe.sync_info.on_wait) if probe.sync_info else []
        for i in range(0, len(waits), MAX_DRAIN_WAITS):
            chunk = waits[i : i + MAX_DRAIN_WAITS]
            nop = nc.sync.nop(nofuse=True, hint="drain_wait")
            nop.ins.sync_info = mybir.SyncInfo(on_wait=chunk, on_update=[])
        nc.sync.drain()

        nc.all_engine_barrier()
        popped = nc._tile_sem_poison_stack.pop()
        assert popped is self._sem_poison
        nc.clear_and_free_semaphores(list(self.sems.allocated().values()))
        nc.all_engine_barrier()

    tile_mod.TileContext._drain_and_barrier = _drain_and_barrier
    tile_mod.TileContext._drain_patched = True


def _build_module():
    import concourse.bass as bass
    import concourse.tile as tile
    from concourse import mybir
    from concourse.masks import make_identity

    _patch_drain()
    f16 = mybir.dt.float16
    f32 = mybir.dt.float32
    EXP = mybir.ActivationFunctionType.Exp

    nc = bass.Bass()
    x_in = nc.dram_tensor("x", [TOK, C], f16, kind="ExternalInput")
    r_in = nc.dram_tensor("r", [C, H], f16, kind="ExternalInput")
    # pooled^T per (batch, c-partition, c-chunk, head)
    s_out = nc.dram_tensor("s", [BPC, 128, 4, H], f32, kind="ExternalOutput")
    # Z partials per (batch, head, chunk-within-batch)
    z_out = nc.dram_tensor("z", [BPC, 32, CPB], f32, kind="ExternalOutput")

    x_whole = x_in.rearrange("(a j p) f -> a p j f", a=NCHUNK, j=4, p=128)
    x_piece = x_in.rearrange("(a j p) f -> a j p f", a=NCHUNK, j=4, p=128)
    r_src = r_in.rearrange("(q p) h -> p q h", p=128)

    with tile.TileContext(nc) as tc:
        with (
            tc.tile_pool(name="xpool", bufs=1) as xpool,
            tc.tile_pool(name="consts", bufs=1) as consts,
            tc.tile_pool(name="xtpool", bufs=2) as xtpool,
            tc.tile_pool(name="epool", bufs=2) as epool,
            tc.tile_pool(name="etpool", bufs=2) as etpool,
            tc.tile_pool(name="zpool", bufs=1) as zpool,
            tc.tile_pool(name="opool", bufs=1) as opool,
            tc.tile_pool(name="pt", bufs=4, space="PSUM") as pt_pool,
            tc.tile_pool(name="pst", bufs=2, space="PSUM") as pst_pool,
            tc.tile_pool(name="pz", bufs=1, space="PSUM") as pz_pool,
            tc.tile_pool(name="pp", bufs=1, space="PSUM") as pp_pool,
        ):
            r_sb = consts.tile([128, 4, H], f16)

            # x loads: early chunks in 128KB j-pieces, alternating HWDGE
            # issue engines (SP / ACT) so the first tile lands ASAP
            x_sb = []
            ndma = 0

            def dma_eng():
                nonlocal ndma
                ndma += 1
                return nc.sync if ndma % 2 else nc.scalar

            for a in range(NCHUNK):
                t = xpool.tile([128, 4, C], f16, tag=f"x{a}", name=f"x{a}")
                x_sb.append(t)
            for a in range(NSPLIT):
                for j in range(4):
                    dma_eng().dma_start(out=x_sb[a][:, j, :], in_=x_piece[a, j])
            for a in range(NSPLIT, NCHUNK):
                dma_eng().dma_start(out=x_sb[a], in_=x_whole[a])
            nc.sync.dma_start(out=r_sb, in_=r_src)

            ident = consts.tile([128, 128], f16)
            make_identity(nc, ident)

            # HAM pre-heat: a contiguous chain of fat N=512 matmuls keeps
            # the PE array streaming for >3.4us while the first x pieces
            # land, so the clock gate is already 8/8 when real work starts
            heat_src = consts.tile([128, 512], f16)
            nc.vector.memset(heat_src, 0.001)
            ht = pt_pool.tile([128, 512], f32, tag="pt", name="heat")
            for k in range(11):
                nc.tensor.matmul(
                    ht, ident, heat_src, start=(k == 0), stop=(k == 10)
                )

            ones = consts.tile([128, 1], f16)
            nc.vector.memset(ones, 1.0)
            # Z partials per (32 (j,h) rows, batch, chunk) in one PSUM bank
            pz = pz_pool.tile([32, BPC, CPB], f32, tag="pz", name="pz")
            # pooled^T accumulator: [c128, b, q, h] single PSUM bank
            pp = pp_pool.tile([128, BPC, 4, H], f32, tag="pp", name="pp")

            ncopy = 0

            def alt_copy(dst, src):
                nonlocal ncopy
                # PSUM->SBUF drains: DVE takes 2 of 3 (ACT also runs exp)
                if ncopy % 3 != 2:
                    nc.vector.tensor_copy(dst, src)
                else:
                    nc.scalar.copy(dst, src)
                ncopy += 1

            xts = {}
            es = {}
            ets = {}

            def stage_T(i):
                """transpose chunk i: per q, 4 identity MMs + PSUM drain."""
                xts[i] = []
                for q in range(4):
                    pt = pt_pool.tile([128, 512], f32, tag="pt", name=f"pt{i}_{q}")
                    for j in range(4):
                        # one accumulation group per pt bank: the 4 matmuls
                        # write disjoint column ranges (overwrite semantics),
                        # avoiding per-MM group micro-idles on the PE
                        nc.tensor.matmul(
                            pt[:, j * 128 : (j + 1) * 128],
                            x_sb[i][:, j, q * 128 : (q + 1) * 128],
                            ident,
                            start=(j == 0),
                            stop=(j == 3),
                        )
                    xt = xtpool.tile(
                        [128, 512], f16, tag=f"xt{q}", name=f"xt{i}_{q}"
                    )
                    alt_copy(xt, pt)
                    xts[i].append(xt)

            def stage_S(i):
                """scores chunk i, transposed form: per t-block j a 4-matmul
                chain over q with the X^T block as FWL weights and R_q as the
                8-col moving operand -> pst[:, j, :] = scores[t, h].  One ACT
                exp turns the whole tile into E^T directly (no E transpose
                pass), and one tiny matmul against a ones vector forms the
                per-chunk Z row-sums."""
                b, g = divmod(i, CPB)
                pst = pst_pool.tile([128, 4, H], f32, tag="pst", name=f"pst{i}")
                for j in range(4):
                    for q in range(4):
                        nc.tensor.matmul(
                            pst[:, j, :],
                            xts[i][q][:, j * 128 : (j + 1) * 128],
                            r_sb[:, q, :],
                            start=(q == 0),
                            stop=(q == 3),
                        )
                del xts[i]
                et = etpool.tile([128, 4, H], f16, tag="et", name=f"et{i}")
                nc.scalar.activation(out=et, in_=pst, func=EXP)
                es[i] = et
                nc.tensor.matmul(
                    pz[:, b, g : g + 1],
                    et,
                    ones,
                    start=True,
                    stop=True,
                )

            def stage_P(i):
                """pooled^T accumulation for chunk i (E^T comes from exp)."""
                b, g = divmod(i, CPB)
                et = es.pop(i)
                first = g == 0
                last = g == CPB - 1
                for q in range(4):
                    for j in range(4):
                        nc.tensor.matmul(
                            pp[:, b, q, :],
                            x_sb[i][:, j, q * 128 : (q + 1) * 128],
                            et[:, j, :],
                            start=(first and q == 0 and j == 0),
                            stop=(last and q == 3 and j == 3),
                        )

            def emit_s(b):
                so = opool.tile([128, 4, H], f32, tag=f"so{b}", name=f"so{b}")
                nc.vector.tensor_copy(so, pp[:, b])
                nc.scalar.dma_start(out=s_out[b], in_=so)

            def emit_z(b):
                zo = opool.tile([32, CPB], f32, tag=f"zo{b}", name=f"zo{b}")
                nc.vector.tensor_copy(zo, pz[:, b])
                nc.scalar.dma_start(out=z_out[b], in_=zo)

            for i in range(NCHUNK + 2):
                if i < NCHUNK:
                    stage_T(i)
                if 1 <= i <= NCHUNK:
                    stage_S(i - 1)
                if i == NCHUNK:
                    emit_z(1)
                if 2 <= i:
                    stage_P(i - 2)
                    if i - 2 == CPB - 1:
                        emit_s(0)
                        emit_z(0)
            emit_s(1)

    return nc


def _get_module():
    if "nc" not in _cached:
        _cached["nc"] = _build_module()
    return _cached["nc"]


def _host_prep(cls, qkv_w):
    scale = HD ** -0.5
    c = cls.reshape(C).astype(np.float64)
    Wq = qkv_w[:C].astype(np.float64)
    Wk = qkv_w[C : 2 * C].astype(np.float64)
    q = Wq @ c
    qh = q.reshape(H, HD)
    Wkh = Wk.reshape(H, HD, C)
    R = (scale * np.einsum("hdc,hd->ch", Wkh, qh)).astype(np.float16)
    k0 = Wk @ c
    score0 = scale * np.einsum("hd,hd->h", qh, k0.reshape(H, HD))
    e0 = np.exp(score0)
    return R, e0


def kernel(x, cls, qkv_w, proj_w, proj_b):
    from concourse.bass_utils import run_bass_kernel_spmd

    x = np.asarray(x, dtype=np.float32)
    cls = np.asarray(cls, dtype=np.float32)
    qkv_w = np.asarray(qkv_w, dtype=np.float32)
    proj_w = np.asarray(proj_w, dtype=np.float32)
    proj_b = np.asarray(proj_b, dtype=np.float32)

    R, e0 = _host_prep(cls, qkv_w)
    Wv = qkv_w[2 * C :]

    x16 = np.ascontiguousarray(x.reshape(B * N, C).astype(np.float16))
    nc = _get_module()
    in_maps = [
        {"x": x16[i * TOK : (i + 1) * TOK], "r": R}
        for i in range(NCORES)
    ]
    res = run_bass_kernel_spmd(nc, in_maps, list(range(NCORES)))
    _cached["last_results"] = res

    s_parts = []
    z_parts = []
    for i in range(NCORES):
        s_dev = res.results[i]["s"]          # [BPC, 128, 4, H]
        z_dev = res.results[i]["z"]          # [BPC, 32, CPB]
        s_parts.append(np.transpose(s_dev, (0, 3, 2, 1)).reshape(BPC, H, C))
        z_parts.append(z_dev.reshape(BPC, 4, H, CPB).sum(axis=(1, 3)))
    s_dev = np.concatenate(s_parts, axis=0)  # [B, H, C]
    z_dev = np.concatenate(z_parts, axis=0)  # [B, H]

    # add the CLS token's own contribution, normalize, head-mix + proj
    cf = cls.reshape(C)
    s_full = s_dev + (e0[:, None] * cf[None, :]).astype(np.float32)[None]
    z_full = z_dev + e0.astype(np.float32)[None]
    v = s_full / z_full[:, :, None]
    o = np.einsum("hdc,bhc->bhd", Wv.reshape(H, HD, C), v).reshape(B, C)
    y = o @ proj_w.T + proj_b
    return y.astype(np.float32)


# revision 33
# speedup vs baseline: 1.0067x; 1.0067x over previous
"""AttentionalJoin kernel for 8 Trainium2 NeuronCores.

Math: the reference builds full (M x M) self-attention over M = N+1 tokens
(CLS prepended) but returns only the CLS row of the projected output.  Only
the CLS query survives, so attention collapses to a softmax-weighted token
pooling:

    q       = Wq @ cls                       (per head h: q_h)
    score_t = scale * q_h . (Wk x_t)_h  =  x_t . R[:, h],   R = scale*Wk_h^T q_h
    p       = softmax over the M tokens (scores bounded ~[-6, 6]; no max-sub)
    pooled_h = sum_t p_t x_t                 (linearity: project AFTER pooling)
    out     = proj( concat_h Wv_h pooled_h ) + proj_b

Device pipeline, software-pipelined at depth 3 over 512-token chunks:
  T(i)   : 16 identity matmuls build X^T per c-chunk (x-block as fp16 FWL
           weights, N=128); PSUM->SBUF drains split 2:1 between DVE and ACT
  S(i-1) : scores TRANSPOSED: per t-block a 4-matmul chain over c-chunks
           with the X^T block as weights and R_q as the 8-col moving
           operand -> [t,h] in PSUM; one ACT exp yields E^T directly (no
           E-transpose pass) and one tiny matmul against a ones vector
           forms the per-chunk Z partition sums
  P(i-2) : 16 matmuls x_block.T @ E^T -> pooled^T [c128, h], one PSUM
           accumulation group per batch across all its chunks
x streams from HBM once in fp16 (rel err ~3e-4); the first chunks are
DMA'd in 128KB j-pieces issued alternately from SP/ACT HWDGE so the PE
starts early, and a burst of heater matmuls keeps the PE clock-gate
warming while they land.  The tiny tail (CLS term, 1/Z, head-mix, proj,
bias) runs on host.

Sharding: data-parallel over the batch dim, 2 batches per core.
"""

import numpy as np

H = 8
C = 512
HD = C // H
B = 16
N = 2048
NCORES = 8
BPC = B // NCORES          # batches per core
TOK = BPC * N              # tokens per core (4096)
NCHUNK = TOK // 512        # 512-token chunks per core (8; 4 per batch)
CPB = NCHUNK // BPC        # chunks per batch (4)
NSPLIT = 3                 # leading chunks DMA'd in j-pieces for early start
MAX_DRAIN_WAITS = 1        # this walrus rejects instructions w/ >1 sem wait

_cached = {}


def _patch_drain():
    """The container's walrus codegen rejects instructions carrying more
    than one sem wait ("Too many sync wait commands").  Split extra waits
    onto dedicated same-engine NOPs, which preserves semantics (engine
    queues are in-order)."""
    import concourse.tile as tile_mod
    from concourse import mybir
    from bass_rust import ScopedClock

    if getattr(tile_mod.TileContext, "_drain_patched", False):
        return

    orig_lower = tile_mod.TileContext._lower_ordered_insts

    def _lower_ordered_insts(self, ordered):
        nc = self.nc
        for bbname, insts in ordered.items():
            out = []
            for inst in insts:
                si = inst.sync_info
                if si is not None and si.on_wait and len(si.on_wait) > MAX_DRAIN_WAITS:
                    waits = list(si.on_wait)
                    extra, keep = waits[:-MAX_DRAIN_WAITS], waits[-MAX_DRAIN_WAITS:]
                    for w in extra:
                        nop = mybir.InstNoOp(
                            name=f"waitsplit-{nc.next_id()}",
                            engine=inst.engine,
                            ins=[],
                            outs=[],
                            bass_nofuse=True,
                            sync_info=mybir.SyncInfo(on_wait=[w], on_update=[]),
                            debug=inst.debug,
                        )
                        out.append(nop)
                    inst.sync_info = mybir.SyncInfo(
                        on_wait=keep, on_update=list(si.on_update)
                    )
                out.append(inst)
            ordered[bbname] = out
        return orig_lower(self, ordered)

    tile_mod.TileContext._lower_ordered_insts = _lower_ordered_insts

    def _drain_and_barrier(self, tick_clock, wait_clock):
        nc = self.nc
        probe = mybir.InstNoOp(
            name=f"drain-wait-probe-{nc.next_id()}",
            engine=mybir.EngineType.SP,
            ins=[],
            outs=[],
        )
        wait_clock.add_sem_waits(probe, ScopedClock({None: tick_clock.global_clock}))
        waits = list(probe.sync_info.on_wait) if probe.sync_info else []
        for i in range(0, len(waits), MAX_DRAIN_WAITS):
            chunk = waits[i : i + MAX_DRAIN_WAITS]
            nop = nc.sync.nop(nofuse=True, hint="drain_wait")
            nop.ins.sync_info = mybir.SyncInfo(on_wait=chunk, on_update=[])
        nc.sync.drain()

        nc.all_engine_barrier()
        popped = nc._tile_sem_poison_stack.pop()
        assert popped is self._sem_poison
        nc.clear_and_free_semaphores(list(self.sems.allocated().values()))
        nc.all_engine_barrier()

    tile_mod.TileContext._drain_and_barrier = _drain_and_barrier
    tile_mod.TileContext._drain_patched = True


def _build_module():
    import concourse.bass as bass
    import concourse.tile as tile
    from concourse import mybir
    from concourse.masks import make_identity

    _patch_drain()
    f16 = mybir.dt.float16
    f32 = mybir.dt.float32
    EXP = mybir.ActivationFunctionType.Exp

    nc = bass.Bass()
    x_in = nc.dram_tensor("x", [TOK, C], f16, kind="ExternalInput")
    r_in = nc.dram_tensor("r", [C, H], f16, kind="ExternalInput")
    # pooled^T per (batch, c-partition, c-chunk, head)
    s_out = nc.dram_tensor("s", [BPC, 128, 4, H], f32, kind="ExternalOutput")
    # Z partials per (batch, head, chunk-within-batch)
    z_out = nc.dram_tensor("z", [BPC, 32, CPB], f32, kind="ExternalOutput")

    x_whole = x_in.rearrange("(a j p) f -> a p j f", a=NCHUNK, j=4, p=128)
    x_piece = x_in.rearrange("(a j p) f -> a j p f", a=NCHUNK, j=4, p=128)
    r_src = r_in.rearrange("(q p) h -> p q h", p=128)

    with tile.TileContext(nc) as tc:
        with (
            tc.tile_pool(name="xpool", bufs=1) as xpool,
            tc.tile_pool(name="consts", bufs=1) as consts,
            tc.tile_pool(name="xtpool", bufs=2) as xtpool,
            tc.tile_pool(name="epool", bufs=2) as epool,
            tc.tile_pool(name="etpool", bufs=2) as etpool,
            tc.tile_pool(name="zpool", bufs=1) as zpool,
            tc.tile_pool(name="opool", bufs=1) as opool,
            tc.tile_pool(name="pt", bufs=4, space="PSUM") as pt_pool,
            tc.tile_pool(name="pst", bufs=2, space="PSUM") as pst_pool,
            tc.tile_pool(name="pz", bufs=1, space="PSUM") as pz_pool,
            tc.tile_pool(name="pp", bufs=1, space="PSUM") as pp_pool,
        ):
            r_sb = consts.tile([128, 4, H], f16)

            # x loads: early chunks in 128KB j-pieces, alternating HWDGE
            # issue engines (SP / ACT) so the first tile lands ASAP
            x_sb = []
            ndma = 0

            def dma_eng():
                nonlocal ndma
                ndma += 1
                return nc.sync if ndma % 2 else nc.scalar

            for a in range(NCHUNK):
                t = xpool.tile([128, 4, C], f16, tag=f"x{a}", name=f"x{a}")
                x_sb.append(t)
            for a in range(NSPLIT):
                for j in range(4):
                    dma_eng().dma_start(out=x_sb[a][:, j, :], in_=x_piece[a, j])
            for a in range(NSPLIT, NCHUNK):
                dma_eng().dma_start(out=x_sb[a], in_=x_whole[a])
            nc.sync.dma_start(out=r_sb, in_=r_src)

            ident = consts.tile([128, 128], f16)
            make_identity(nc, ident)

            # HAM heaters: keep the PE streaming while the first x pieces
            # land so the clock gate is at 8/8 when real work starts
            nheat = 0

            def heat(n):
                nonlocal nheat
                ht = pt_pool.tile([128, 128], f32, tag="pt", name=f"heat{nheat}")
                for k in range(n):
                    nc.tensor.matmul(ht, ident, ident, start=(k == 0), stop=(k == n - 1))
                    nheat += 1

            heat(14)

            ones = consts.tile([128, 1], f16)
            nc.vector.memset(ones, 1.0)
            # Z partials per (32 (j,h) rows, batch, chunk) in one PSUM bank
            pz = pz_pool.tile([32, BPC, CPB], f32, tag="pz", name="pz")
            # pooled^T accumulator: [c128, b, q, h] single PSUM bank
            pp = pp_pool.tile([128, BPC, 4, H], f32, tag="pp", name="pp")

            ncopy = 0

            def alt_copy(dst, src):
                nonlocal ncopy
                # PSUM->SBUF drains: DVE takes 2 of 3 (ACT also runs exp)
                if ncopy % 3 != 2:
                    nc.vector.tensor_copy(dst, src)
                else:
                    nc.scalar.copy(dst, src)
                ncopy += 1

            xts = {}
            es = {}
            ets = {}

            def stage_T(i):
                """transpose chunk i: per q, 4 identity MMs + PSUM drain."""
                xts[i] = []
                for q in range(4):
                    pt = pt_pool.tile([128, 512], f32, tag="pt", name=f"pt{i}_{q}")
                    for j in range(4):
                        # one accumulation group per pt bank: the 4 matmuls
                        # write disjoint column ranges (overwrite semantics),
                        # avoiding per-MM group micro-idles on the PE
                        nc.tensor.matmul(
                            pt[:, j * 128 : (j + 1) * 128],
                            x_sb[i][:, j, q * 128 : (q + 1) * 128],
                            ident,
                            start=(j == 0),
                            stop=(j == 3),
                        )
                    xt = xtpool.tile(
                        [128, 512], f16, tag=f"xt{q}", name=f"xt{i}_{q}"
                    )
                    alt_copy(xt, pt)
                    xts[i].append(xt)

            def stage_S(i):
                """scores chunk i, transposed form: per t-block j a 4-matmul
                chain over q with the X^T block as FWL weights and R_q as the
                8-col moving operand -> pst[:, j, :] = scores[t, h].  One ACT
                exp turns the whole tile into E^T directly (no E transpose
                pass), and one tiny matmul against a ones vector forms the
                per-chunk Z row-sums."""
                b, g = divmod(i, CPB)
                pst = pst_pool.tile([128, 4, H], f32, tag="pst", name=f"pst{i}")
                for j in range(4):
                    for q in range(4):
                        nc.tensor.matmul(
                            pst[:, j, :],
                            xts[i][q][:, j * 128 : (j + 1) * 128],
                            r_sb[:, q, :],
                            start=(q == 0),
                            stop=(q == 3),
                        )
                del xts[i]
                et = etpool.tile([128, 4, H], f16, tag="et", name=f"et{i}")
                nc.scalar.activation(out=et, in_=pst, func=EXP)
                es[i] = et
                nc.tensor.matmul(
                    pz[:, b, g : g + 1],
                    et,
                    ones,
                    start=True,
                    stop=True,
                )

            def stage_P(i):
                """pooled^T accumulation for chunk i (E^T comes from exp)."""
                b, g = divmod(i, CPB)
                et = es.pop(i)
                first = g == 0
                last = g == CPB - 1
                for q in range(4):
                    for j in range(4):
                        nc.tensor.matmul(
                            pp[:, b, q, :],
                            x_sb[i][:, j, q * 128 : (q + 1) * 128],
                            et[:, j, :],
                            start=(first and q == 0 and j == 0),
                            stop=(last and q == 3 and j == 3),
                        )

            def emit_s(b):
                so = opool.tile([128, 4, H], f32, tag=f"so{b}", name=f"so{b}")
                nc.vector.tensor_copy(so, pp[:, b])
                nc.scalar.dma_start(out=s_out[b], in_=so)

            def emit_z(b):
                zo = opool.tile([32, CPB], f32, tag=f"zo{b}", name=f"zo{b}")
                nc.vector.tensor_copy(zo, pz[:, b])
                nc.scalar.dma_start(out=z_out[b], in_=zo)

            for i in range(NCHUNK + 2):
                if i < NCHUNK:
                    stage_T(i)
                if 1 <= i <= NCHUNK:
                    stage_S(i - 1)
                if i == CPB:
                    emit_z(0)
                if i == NCHUNK:
                    emit_z(1)
                if 2 <= i:
                    stage_P(i - 2)
                    if i - 2 == CPB - 1:
                        emit_s(0)
            emit_s(1)

    return nc


def _get_module():
    if "nc" not in _cached:
        _cached["nc"] = _build_module()
    return _cached["nc"]


def _host_prep(cls, qkv_w):
    scale = HD ** -0.5
    c = cls.reshape(C).astype(np.float64)
    Wq = qkv_w[:C].astype(np.float64)
    Wk = qkv_w[C : 2 * C].astype(np.float64)
    q = Wq @ c
    qh = q.reshape(H, HD)
    Wkh = Wk.reshape(H, HD, C)
    R = (scale * np.einsum("hdc,hd->ch", Wkh, qh)).astype(np.float16)
    k0 = Wk @ c
    score0 = scale * np.einsum("hd,hd->h", qh, k0.reshape(H, HD))
    e0 = np.exp(score0)
    return R, e0


def kernel(x, cls, qkv_w, proj_w, proj_b):
    from concourse.bass_utils import run_bass_kernel_spmd

    x = np.asarray(x, dtype=np.float32)
    cls = np.asarray(cls, dtype=np.float32)
    qkv_w = np.asarray(qkv_w, dtype=np.float32)
    proj_w = np.asarray(proj_w, dtype=np.float32)
    proj_b = np.asarray(proj_b, dtype=np.float32)

    R, e0 = _host_prep(cls, qkv_w)
    Wv = qkv_w[2 * C :]

    x16 = np.ascontiguousarray(x.reshape(B * N, C).astype(np.float16))
    nc = _get_module()
    in_maps = [
        {"x": x16[i * TOK : (i + 1) * TOK], "r": R}
        for i in range(NCORES)
    ]
    res = run_bass_kernel_spmd(nc, in_maps, list(range(NCORES)))
    _cached["last_results"] = res

    s_parts = []
    z_parts = []
    for i in range(NCORES):
        s_dev = res.results[i]["s"]          # [BPC, 128, 4, H]
        z_dev = res.results[i]["z"]          # [BPC, 32, CPB]
        s_parts.append(np.transpose(s_dev, (0, 3, 2, 1)).reshape(BPC, H, C))
        z_parts.append(z_dev.reshape(BPC, 4, H, CPB).sum(axis=(1, 3)))
    s_dev = np.concatenate(s_parts, axis=0)  # [B, H, C]
    z_dev = np.concatenate(z_parts, axis=0)  # [B, H]

    # add the CLS token's own contribution, normalize, head-mix + proj
    cf = cls.reshape(C)
    s_full = s_dev + (e0[:, None] * cf[None, :]).astype(np.float32)[None]
    z_full = z_dev + e0.astype(np.float32)[None]
    v = s_full / z_full[:, :, None]
    o = np.einsum("hdc,bhc->bhd", Wv.reshape(H, HD, C), v).reshape(B, C)
    y = o @ proj_w.T + proj_b
    return y.astype(np.float32)


# revision 35
# speedup vs baseline: 1.1208x; 1.1133x over previous
"""AttentionalJoin kernel for 8 Trainium2 NeuronCores.

Math: the reference builds full (M x M) self-attention over M = N+1 tokens
(CLS prepended) but returns only the CLS row of the projected output.  Only
the CLS query survives, so attention collapses to a softmax-weighted token
pooling:

    q       = Wq @ cls                       (per head h: q_h)
    score_t = scale * q_h . (Wk x_t)_h  =  x_t . R[:, h],   R = scale*Wk_h^T q_h
    p       = softmax over the M tokens (scores bounded ~[-6, 6]; no max-sub)
    pooled_h = sum_t p_t x_t                 (linearity: project AFTER pooling)
    out     = proj( concat_h Wv_h pooled_h ) + proj_b

Device pipeline, software-pipelined at depth 3 over 512-token chunks:
  T(i)   : 16 identity matmuls build X^T per c-chunk (x-block as fp16 FWL
           weights, N=128); PSUM->SBUF drains split 2:1 between DVE and ACT
  S(i-1) : scores TRANSPOSED: per t-block a 4-matmul chain over c-chunks
           with the X^T block as weights and R_q as the 8-col moving
           operand -> [t,h] in PSUM; one ACT exp yields E^T directly (no
           E-transpose pass) and one tiny matmul against a ones vector
           forms the per-chunk Z partition sums
  P(i-2) : 16 matmuls x_block.T @ E^T -> pooled^T [c128, h], one PSUM
           accumulation group per batch across all its chunks
x streams from HBM once in fp16 (rel err ~3e-4); the first chunks are
DMA'd in 128KB j-pieces issued alternately from SP/ACT HWDGE so the PE
starts early, and a burst of heater matmuls keeps the PE clock-gate
warming while they land.  The tiny tail (CLS term, 1/Z, head-mix, proj,
bias) runs on host.

Sharding: data-parallel over the batch dim, 2 batches per core.
"""

import numpy as np

H = 8
C = 512
HD = C // H
B = 16
N = 2048
NCORES = 8
BPC = B // NCORES          # batches per core
TOK = BPC * N              # tokens per core (4096)
NCHUNK = TOK // 512        # 512-token chunks per core (8; 4 per batch)
CPB = NCHUNK // BPC        # chunks per batch (4)
NSPLIT = 3                 # leading chunks DMA'd in j-pieces for early start
MAX_DRAIN_WAITS = 1        # this walrus rejects instructions w/ >1 sem wait

_cached = {}


def _patch_drain():
    """The container's walrus codegen rejects instructions carrying more
    than one sem wait ("Too many sync wait commands").  Split extra waits
    onto dedicated same-engine NOPs, which preserves semantics (engine
    queues are in-order)."""
    import concourse.tile as tile_mod
    from concourse import mybir
    from bass_rust import ScopedClock

    if getattr(tile_mod.TileContext, "_drain_patched", False):
        return

    orig_lower = tile_mod.TileContext._lower_ordered_insts

    def _lower_ordered_insts(self, ordered):
        nc = self.nc
        for bbname, insts in ordered.items():
            out = []
            for inst in insts:
                si = inst.sync_info
                if si is not None and si.on_wait and len(si.on_wait) > MAX_DRAIN_WAITS:
                    waits = list(si.on_wait)
                    extra, keep = waits[:-MAX_DRAIN_WAITS], waits[-MAX_DRAIN_WAITS:]
                    for w in extra:
                        nop = mybir.InstNoOp(
                            name=f"waitsplit-{nc.next_id()}",
                            engine=inst.engine,
                            ins=[],
                            outs=[],
                            bass_nofuse=True,
                            sync_info=mybir.SyncInfo(on_wait=[w], on_update=[]),
                            debug=inst.debug,
                        )
                        out.append(nop)
                    inst.sync_info = mybir.SyncInfo(
                        on_wait=keep, on_update=list(si.on_update)
                    )
                out.append(inst)
            ordered[bbname] = out
        return orig_lower(self, ordered)

    tile_mod.TileContext._lower_ordered_insts = _lower_ordered_insts

    def _drain_and_barrier(self, tick_clock, wait_clock):
        nc = self.nc
        probe = mybir.InstNoOp(
            name=f"drain-wait-probe-{nc.next_id()}",
            engine=mybir.EngineType.SP,
            ins=[],
            outs=[],
        )
        wait_clock.add_sem_waits(probe, ScopedClock({None: tick_clock.global_clock}))
        waits = list(probe.sync_info.on_wait) if probe.sync_info else []
        for i in range(0, len(waits), MAX_DRAIN_WAITS):
            chunk = waits[i : i + MAX_DRAIN_WAITS]
            nop = nc.sync.nop(nofuse=True, hint="drain_wait")
            nop.ins.sync_info = mybir.SyncInfo(on_wait=chunk, on_update=[])
        nc.sync.drain()

        nc.all_engine_barrier()
        popped = nc._tile_sem_poison_stack.pop()
        assert popped is self._sem_poison
        nc.clear_and_free_semaphores(list(self.sems.allocated().values()))
        nc.all_engine_barrier()

    tile_mod.TileContext._drain_and_barrier = _drain_and_barrier
    tile_mod.TileContext._drain_patched = True


def _build_module():
    import concourse.bass as bass
    import concourse.tile as tile
    from concourse import mybir
    from concourse.masks import make_identity

    _patch_drain()
    f16 = mybir.dt.float16
    f32 = mybir.dt.float32
    EXP = mybir.ActivationFunctionType.Exp

    nc = bass.Bass()
    x_in = nc.dram_tensor("x", [TOK, C], f16, kind="ExternalInput")
    r_in = nc.dram_tensor("r", [C, H], f16, kind="ExternalInput")
    # pooled^T per (batch, c-partition, c-chunk, head)
    s_out = nc.dram_tensor("s", [BPC, 128, 4, H], f32, kind="ExternalOutput")
    # Z partials per (batch, head, chunk-within-batch)
    z_out = nc.dram_tensor("z", [BPC, 32, CPB], f32, kind="ExternalOutput")

    x_whole = x_in.rearrange("(a j p) f -> a p j f", a=NCHUNK, j=4, p=128)
    x_piece = x_in.rearrange("(a j p) f -> a j p f", a=NCHUNK, j=4, p=128)
    r_src = r_in.rearrange("(q p) h -> p q h", p=128)

    with tile.TileContext(nc) as tc:
        with (
            tc.tile_pool(name="xpool", bufs=1) as xpool,
            tc.tile_pool(name="consts", bufs=1) as consts,
            tc.tile_pool(name="xtpool", bufs=3) as xtpool,
            tc.tile_pool(name="epool", bufs=3) as epool,
            tc.tile_pool(name="etpool", bufs=3) as etpool,
            tc.tile_pool(name="zpool", bufs=1) as zpool,
            tc.tile_pool(name="opool", bufs=1) as opool,
            tc.tile_pool(name="pt", bufs=4, space="PSUM") as pt_pool,
            tc.tile_pool(name="pst", bufs=2, space="PSUM") as pst_pool,
            tc.tile_pool(name="pz", bufs=1, space="PSUM") as pz_pool,
            tc.tile_pool(name="pp", bufs=1, space="PSUM") as pp_pool,
        ):
            r_sb = consts.tile([128, 4, H], f16)
            nc.sync.dma_start(out=r_sb, in_=r_src)

            # x loads: early chunks in 128KB j-pieces, alternating HWDGE
            # issue engines (SP / ACT) so the first tile lands ASAP
            x_sb = []
            ndma = 0

            def dma_eng():
                nonlocal ndma
                ndma += 1
                return nc.sync if ndma % 2 else nc.scalar

            for a in range(NCHUNK):
                t = xpool.tile([128, 4, C], f16, tag=f"x{a}", name=f"x{a}")
                x_sb.append(t)
            for a in range(NSPLIT):
                for j in range(4):
                    dma_eng().dma_start(out=x_sb[a][:, j, :], in_=x_piece[a, j])
            for a in range(NSPLIT, NCHUNK):
                dma_eng().dma_start(out=x_sb[a], in_=x_whole[a])

            ident = consts.tile([128, 128], f16)
            make_identity(nc, ident)

            # HAM heaters: keep the PE streaming while the first x pieces
            # land so the clock gate is at 8/8 when real work starts
            nheat = 0

            def heat(n):
                nonlocal nheat
                ht = pt_pool.tile([128, 128], f32, tag="pt", name=f"heat{nheat}")
                for k in range(n):
                    nc.tensor.matmul(ht, ident, ident, start=(k == 0), stop=(k == n - 1))
                    nheat += 1

            heat(14)

            ones = consts.tile([128, 1], f16)
            nc.vector.memset(ones, 1.0)
            # Z partials per (32 (j,h) rows, batch, chunk) in one PSUM bank
            pz = pz_pool.tile([32, BPC, CPB], f32, tag="pz", name="pz")
            # pooled^T accumulator: [c128, b, q, h] single PSUM bank
            pp = pp_pool.tile([128, BPC, 4, H], f32, tag="pp", name="pp")

            ncopy = 0

            def alt_copy(dst, src):
                nonlocal ncopy
                # PSUM->SBUF drains: DVE takes 2 of 3 (ACT also runs exp)
                if ncopy % 3 != 2:
                    nc.vector.tensor_copy(dst, src)
                else:
                    nc.scalar.copy(dst, src)
                ncopy += 1

            xts = {}
            es = {}
            ets = {}

            def stage_T(i):
                """transpose chunk i: per q, 4 identity MMs + PSUM drain."""
                xts[i] = []
                for q in range(4):
                    pt = pt_pool.tile([128, 512], f32, tag="pt", name=f"pt{i}_{q}")
                    for j in range(4):
                        # one accumulation group per pt bank: the 4 matmuls
                        # write disjoint column ranges (overwrite semantics),
                        # avoiding per-MM group micro-idles on the PE
                        nc.tensor.matmul(
                            pt[:, j * 128 : (j + 1) * 128],
                            x_sb[i][:, j, q * 128 : (q + 1) * 128],
                            ident,
                            start=(j == 0),
                            stop=(j == 3),
                        )
                    xt = xtpool.tile(
                        [128, 512], f16, tag=f"xt{q}", name=f"xt{i}_{q}"
                    )
                    alt_copy(xt, pt)
                    xts[i].append(xt)

            def stage_S(i):
                """scores chunk i, transposed form: per t-block j a 4-matmul
                chain over q with the X^T block as FWL weights and R_q as the
                8-col moving operand -> pst[:, j, :] = scores[t, h].  One ACT
                exp turns the whole tile into E^T directly (no E transpose
                pass), and one tiny matmul against a ones vector forms the
                per-chunk Z row-sums."""
                b, g = divmod(i, CPB)
                pst = pst_pool.tile([128, 4, H], f32, tag="pst", name=f"pst{i}")
                for j in range(4):
                    for q in range(4):
                        nc.tensor.matmul(
                            pst[:, j, :],
                            xts[i][q][:, j * 128 : (j + 1) * 128],
                            r_sb[:, q, :],
                            start=(q == 0),
                            stop=(q == 3),
                        )
                del xts[i]
                et = etpool.tile([128, 4, H], f16, tag="et", name=f"et{i}")
                nc.scalar.activation(out=et, in_=pst, func=EXP)
                es[i] = et
                nc.tensor.matmul(
                    pz[:, b, g : g + 1],
                    et,
                    ones,
                    start=True,
                    stop=True,
                )

            def stage_P(i):
                """pooled^T accumulation for chunk i (E^T comes from exp)."""
                b, g = divmod(i, CPB)
                et = es.pop(i)
                first = g == 0
                last = g == CPB - 1
                for q in range(4):
                    for j in range(4):
                        nc.tensor.matmul(
                            pp[:, b, q, :],
                            x_sb[i][:, j, q * 128 : (q + 1) * 128],
                            et[:, j, :],
                            start=(first and q == 0 and j == 0),
                            stop=(last and q == 3 and j == 3),
                        )

            def emit_out(b):
                so = opool.tile([128, 4, H], f32, tag=f"so{b}", name=f"so{b}")
                nc.vector.tensor_copy(so, pp[:, b])
                nc.scalar.dma_start(out=s_out[b], in_=so)
                zo = opool.tile([32, CPB], f32, tag=f"zo{b}", name=f"zo{b}")
                nc.vector.tensor_copy(zo, pz[:, b])
                nc.scalar.dma_start(out=z_out[b], in_=zo)

            for i in range(NCHUNK + 2):
                if i < NCHUNK:
                    stage_T(i)
                if 1 <= i <= NCHUNK:
                    stage_S(i - 1)
                if 2 <= i:
                    stage_P(i - 2)
                    if i - 2 == CPB - 1:
                        emit_out(0)
            emit_out(1)

    return nc


def _get_module():
    if "nc" not in _cached:
        _cached["nc"] = _build_module()
    return _cached["nc"]


def _host_prep(cls, qkv_w):
    scale = HD ** -0.5
    c = cls.reshape(C).astype(np.float64)
    Wq = qkv_w[:C].astype(np.float64)
    Wk = qkv_w[C : 2 * C].astype(np.float64)
    q = Wq @ c
    qh = q.reshape(H, HD)
    Wkh = Wk.reshape(H, HD, C)
    R = (scale * np.einsum("hdc,hd->ch", Wkh, qh)).astype(np.float16)
    k0 = Wk @ c
    score0 = scale * np.einsum("hd,hd->h", qh, k0.reshape(H, HD))
    e0 = np.exp(score0)
    return R, e0


def kernel(x, cls, qkv_w, proj_w, proj_b):
    from concourse.bass_utils import run_bass_kernel_spmd

    x = np.asarray(x, dtype=np.float32)
    cls = np.asarray(cls, dtype=np.float32)
    qkv_w = np.asarray(qkv_w, dtype=np.float32)
    proj_w = np.asarray(proj_w, dtype=np.float32)
    proj_b = np.asarray(proj_b, dtype=np.float32)

    R, e0 = _host_prep(cls, qkv_w)
    Wv = qkv_w[2 * C :]

    x16 = np.ascontiguousarray(x.reshape(B * N, C).astype(np.float16))
    nc = _get_module()
    in_maps = [
        {"x": x16[i * TOK : (i + 1) * TOK], "r": R}
        for i in range(NCORES)
    ]
    res = run_bass_kernel_spmd(nc, in_maps, list(range(NCORES)))
    _cached["last_results"] = res

    s_parts = []
    z_parts = []
    for i in range(NCORES):
        s_dev = res.results[i]["s"]          # [BPC, 128, 4, H]
        z_dev = res.results[i]["z"]          # [BPC, 32, CPB]
        s_parts.append(np.transpose(s_dev, (0, 3, 2, 1)).reshape(BPC, H, C))
        z_parts.append(z_dev.reshape(BPC, 4, H, CPB).sum(axis=(1, 3)))
    s_dev = np.concatenate(s_parts, axis=0)  # [B, H, C]
    z_dev = np.concatenate(z_parts, axis=0)  # [B, H]

    # add the CLS token's own contribution, normalize, head-mix + proj
    cf = cls.reshape(C)
    s_full = s_dev + (e0[:, None] * cf[None, :]).astype(np.float32)[None]
    z_full = z_dev + e0.astype(np.float32)[None]
    v = s_full / z_full[:, :, None]
    o = np.einsum("hdc,bhc->bhd", Wv.reshape(H, HD, C), v).reshape(B, C)
    y = o @ proj_w.T + proj_b
    return y.astype(np.float32)
